# revision 47
# baseline (speedup 1.0000x reference)
"""Trainium2 Bass kernel for DisentangledSelfAttention (DeBERTa-style).

Shapes (hardcoded): B=2, S=2048, D=1024, H=16, Dh=64, MAX_REL=512.

Sharding: 8 cores; core c handles batch b = c//4 and heads h0 = (c%4)*4 .. +4
(tensor-parallel on heads for q/k/v columns and c_proj rows; data-parallel on
batch).

Host<->device traffic is minimized (the axon tunnel is the bottleneck):
  - Each core receives only 1/4 of its batch's transposed hidden states plus
    1/4 of the position table (bf16, ~1.1MB) and HALF of its weight slice
    (bf16, ~1.0MB).  On-device AllGathers reassemble them: hs+postable over
    batch groups [[0..3],[4..7]], weights over pairs [[0,4],[1,5],[2,6],[3,7]]
    (a pair shares the same head slice, so the two halves combine).
  - Compute runs in f32r exactly as before (bf16 operands are upcast on load).
  - The per-core c_proj partial [1024, 2048] is ReduceScattered (add, bf16)
    over the batch group, so each core returns only its 256 output channels
    [256, 2048] bf16; the host transposes/concats.

Math per core (heads are local 0..3):
  qT/kT [256, 2048] = W.T-slice @ hsT (+bias), v [2048, 256] natural.
  scoresT[j,i] = k_h.T q_h + 8*t[clip(i-j+512)] + 8*kp-term, exp'd with
  scale 1/8, then out = (v|1).T @ exp  -> av[65, i], normalized by row 64.
  c2p uses t = qsum @ PTW (Toeplitz band added via overlap-staged TS2 tile
  read with a negative-free-stride add on gpsimd); p2c uses per-(head,jc)
  kp windows in anti-diagonal coords, bounced through DRAM and re-read with
  a skewed (diagonal) DMA access pattern covering all 4 query stripes at
  once, then added on the vector engine (the two identity-select matmuls
  these adds replace cost ~2x the score matmul on this backend).  The kp
  matmul only covers the non-saturated band W in [1536, 2560) of
  PTW[w] = 8*pos_table[clip(2559-w)]; outside it the window is the per-key
  constant k.PTW[1536 or 2559], broadcast-filled by gpsimd tensor_scalar
  (cuts phase-2 tensor-engine output elements by 57%).  Four PSUM
  accumulators run the AV matmuls for all query stripes concurrently.
"""
import os

os.environ.setdefault("NEURON_RT_RESET_CORES", "1")

import numpy as np
import ml_dtypes

import concourse.bass as bass
import concourse.bacc as bacc
import concourse.mybir as mybir
import concourse.tile as tile
from concourse.bass_utils import run_bass_kernel_spmd

F32, BF16, F32R = mybir.dt.float32, mybir.dt.bfloat16, mybir.dt.float32r
F16, I8 = mybir.dt.float16, mybir.dt.int8
NPBF16 = ml_dtypes.bfloat16

B, S, D = 2, 2048, 1024
H, Dh, MAX_REL = 16, 64, 512
NCORES = 8
HPC = H // (NCORES // B)   # heads per core = 4
CLOC = HPC * Dh            # local head-dim columns = 256
WW = 2176                  # kp window width per jc chunk
TSW = 3968                 # c2p staging width
PTWN = 4096                # table rows

# --- packed-input layout (element counts, bf16) ---
HSQ = 256 * S              # hs quarter = 524288
PTQ = 16 * 1024            # raw pos-table quarter (16 of 64 ptT rows) = 16384
AGH = HSQ + PTQ            # per-member stride in gathered hs pack = 540672
NUH = 4 * AGH              # gathered hs pack

WSL = D * CLOC             # one weight slice = 262144
NW = 4 * WSL + 3 * CLOC    # weights + 3 bias slices = 1049344
KW = NW // 2               # per-core half = 524672
OFF_WK, OFF_WV, OFF_WC = WSL, 2 * WSL, 3 * WSL
OFF_B = 4 * WSL


def build_nc(max_phase=9):
    nc = bacc.Bacc("TRN2", target_bir_lowering=False)
    pack = nc.dram_tensor("pack", [AGH + KW], BF16, kind="ExternalInput")
    outP = nc.dram_tensor("outP", [CLOC, S], BF16, kind="ExternalOutput")

    aginh = nc.dram_tensor("aginh", [AGH], BF16, kind="Internal")
    aginw = nc.dram_tensor("aginw", [KW], BF16, kind="Internal")
    Uhs = nc.dram_tensor("Uhs", [NUH], BF16, kind="Internal")
    UW = nc.dram_tensor("UW", [NW], BF16, kind="Internal")
    outPart = nc.dram_tensor("outPart", [D, S], BF16, kind="Internal")
    outRS = nc.dram_tensor("outRS", [CLOC, S], BF16, kind="Internal")

    trev_dram = [nc.dram_tensor(f"trev{h}", [PTWN], F16, kind="Internal")
                 for h in range(HPC)]
    # per-head clip-constant rows: kpc[side, j] = 8*k[j].pt[1023 or 0]
    kpc_dram = [nc.dram_tensor(f"kpc{h}", [2, S], F32, kind="Internal")
                for h in range(HPC)]
    kpwin_dram = [nc.dram_tensor(f"kpwin{h}", [16, 128, WW], BF16,
                                 kind="Internal") for h in range(HPC)]

    with tile.TileContext(nc) as tc:
        with (
            tc.tile_pool(name="consts", bufs=1) as consts,
            tc.tile_pool(name="big", bufs=1) as big,
            tc.tile_pool(name="work", bufs=2) as work,
            tc.tile_pool(name="stage", bufs=1) as stage,
            tc.tile_pool(name="hsst", bufs=2) as hsst,
            tc.tile_pool(name="wst", bufs=2) as wstp,
            tc.tile_pool(name="pp", bufs=4, space="PSUM") as pp,
            tc.tile_pool(name="pav", bufs=1, space="PSUM") as pav,
            nc.allow_low_precision(reason="f32r operand rounding throughout"),
        ):
            # ---- Phase -1: bounce packed inputs, AllGather on device ----
            nc.sync.dma_start(aginh[:], pack[0:AGH])
            nc.sync.dma_start(aginw[:], pack[AGH:AGH + KW])
            nc.gpsimd.collective_compute(
                "AllGather", mybir.AluOpType.bypass,
                replica_groups=[[0, 4], [1, 5], [2, 6], [3, 7]],
                ins=[aginw.ap().opt()], outs=[UW.ap().opt()])
            nc.gpsimd.collective_compute(
                "AllGather", mybir.AluOpType.bypass,
                replica_groups=[[0, 1, 2, 3], [4, 5, 6, 7]],
                ins=[aginh.ap().opt()], outs=[Uhs.ap().opt()])

            # ---- Phase 0: constants / weights / tables (bf16 -> f32r) ----
            WqT_sb = consts.tile([128, 8, CLOC], F32R, name="WqT_sb")
            WkT_sb = consts.tile([128, 8, CLOC], F32R, name="WkT_sb")
            WvT_sb = consts.tile([128, 8, CLOC], F32R, name="WvT_sb")
            for dst, off in ((WqT_sb, 0), (WkT_sb, OFF_WK), (WvT_sb, OFF_WV)):
                wt = wstp.tile([128, 8, CLOC], BF16, name="wt", tag="wt")
                nc.sync.dma_start(
                    wt[:], bass.AP(tensor=UW, offset=off,
                                   ap=[[CLOC, 128], [128 * CLOC, 8], [1, CLOC]]))
                nc.vector.tensor_copy(out=dst[:], in_=wt[:])
            WcT_sb = consts.tile([128, 2, D], F32R, name="WcT_sb")
            wt = wstp.tile([128, 2, D], BF16, name="wtc", tag="wt")
            nc.sync.dma_start(
                wt[:], bass.AP(tensor=UW, offset=OFF_WC,
                               ap=[[D, 128], [128 * D, 2], [1, D]]))
            nc.vector.tensor_copy(out=WcT_sb[:], in_=wt[:])

            # Build PTWT[d, W] = 8*ptT[d, clip(2559-W, 0, 1023)] on device
            # from raw bf16 ptT quarters (saves 96KB/core of transfer):
            # W in [0, 1537) -> const col 1023; [1537, 2560) -> reversed
            # slice; [2560, 4096) -> const col 0.
            PTWT_sb = consts.tile([128, PTWN], F32R, name="PTWT_sb")
            ptst = consts.tile([128, 1600], BF16, name="ptst")
            for k in range(4):
                src = bass.AP(tensor=Uhs, offset=k * AGH + HSQ,
                              ap=[[1024, 16], [1, 1024]])
                nc.sync.dma_start(ptst[16 * k:16 * (k + 1), 0:1024], src)
                nc.sync.dma_start(ptst[64 + 16 * k:64 + 16 * (k + 1), 0:1024],
                                  src)
            pt8 = consts.tile([128, 1024], F32R, name="pt8")
            nc.scalar.activation(
                out=pt8[:], in_=ptst[:, 0:1024],
                func=mybir.ActivationFunctionType.Identity, scale=8.0)
            for a, b, bias_col in ((0, 1024, 1023), (1024, 1537, 1023),
                                   (2560, 3584, 0), (3584, 4096, 0)):
                nc.scalar.activation(
                    out=PTWT_sb[:, a:b], in_=pt8[:, 0:b - a],
                    func=mybir.ActivationFunctionType.Identity,
                    bias=pt8[:, bias_col:bias_col + 1], scale=0.0)
            nc.vector.tensor_copy(
                out=PTWT_sb[:, 1537:2560],
                in_=bass.AP(tensor=pt8.tensor, offset=pt8.offset + 1022,
                            ap=[[1024, 128], [-1, 1023]]))

            bq_sb = consts.tile([128, 2], F32, name="bq_sb")
            bk_sb = consts.tile([128, 2], F32, name="bk_sb")
            bst = consts.tile([128, 2, 2], BF16, name="bst")
            for i, dst in enumerate((bq_sb, bk_sb)):
                nc.sync.dma_start(
                    bst[:, :, i], bass.AP(tensor=UW, offset=OFF_B + i * CLOC,
                                          ap=[[1, 128], [128, 2]]))
                nc.vector.tensor_copy(out=dst[:], in_=bst[:, :, i])
            bv_bc = consts.tile([128, CLOC], F32, name="bv_bc")
            bvst = consts.tile([128, CLOC], BF16, name="bvst")
            nc.sync.dma_start(
                bvst[:], bass.AP(tensor=UW, offset=OFF_B + 2 * CLOC,
                                 ap=[[0, 128], [1, CLOC]]))
            nc.vector.tensor_copy(out=bv_bc[:], in_=bvst[:])

            ones_f = consts.tile([128, 1], F32, name="ones_f")
            nc.vector.memset(ones_f[:], 1.0)
            ones_r = consts.tile([128, 1], F32R, name="ones_r")
            nc.vector.tensor_copy(out=ones_r[:], in_=ones_f[:])
            onesrow_f = consts.tile([1, 64], F32, name="onesrow_f")
            nc.vector.memset(onesrow_f[:], 1.0)
            onesrow_r = consts.tile([1, 64], F32R, name="onesrow_r")
            nc.vector.tensor_copy(out=onesrow_r[:], in_=onesrow_f[:])

            # ---- Phase 1: projections, streaming hs in 256-col chunks ----
            qT_sb = big.tile([128, 2, S], F32R, name="qT_sb")
            kT_sb = big.tile([128, 2, S], F32R, name="kT_sb")
            v_sb = big.tile([128, 16, HPC, 65], F32R, name="v_sb")
            for rc in range(8):
                r0 = rc * 256
                hs_bf = hsst.tile([128, 8, 256], BF16, name="hs_bf", tag="hsbf")
                for k in range(4):
                    nc.sync.dma_start(
                        hs_bf[:, 2 * k:2 * k + 2, :],
                        bass.AP(tensor=Uhs, offset=k * AGH + r0,
                                ap=[[S, 128], [128 * S, 2], [1, 256]]))
                hs_ck = hsst.tile([128, 8, 256], F32R, name="hs_ck", tag="hsck")
                nc.vector.tensor_copy(out=hs_ck[:], in_=hs_bf[:])
                for dst, w_sb, b_sb in ((qT_sb, WqT_sb, bq_sb),
                                        (kT_sb, WkT_sb, bk_sb)):
                    for hh in range(2):
                        ps = pp.tile([128, 512], F32, name="ps_proj", tag="psA")
                        for dc in range(8):
                            nc.tensor.matmul(
                                ps[:, 0:256],
                                w_sb[:, dc, hh * 128:(hh + 1) * 128],
                                hs_ck[:, dc, :],
                                start=(dc == 0), stop=(dc == 7))
                        nc.scalar.activation(
                            out=dst[:, hh, r0:r0 + 256], in_=ps[:, 0:256],
                            func=mybir.ActivationFunctionType.Identity,
                            bias=b_sb[:, hh:hh + 1], scale=1.0)
                for sub in range(2):
                    rr = rc * 2 + sub
                    ps = pp.tile([128, 512], F32, name="ps_v", tag="psA")
                    for dc in range(8):
                        nc.tensor.matmul(
                            ps[:, 0:256], hs_ck[:, dc, sub * 128:(sub + 1) * 128],
                            WvT_sb[:, dc, :], start=(dc == 0), stop=(dc == 7))
                    for h in range(HPC):
                        nc.vector.tensor_tensor(
                            v_sb[:, rr, h, 0:64], ps[:, h * 64:(h + 1) * 64],
                            bv_bc[:, h * 64:(h + 1) * 64], mybir.AluOpType.add)
                        nc.vector.tensor_copy(out=v_sb[:, rr, h, 64:65],
                                              in_=ones_r[:])

            # phase gating for bisection
            PH15 = HPC if max_phase >= 2 else 0
            PH2 = HPC if max_phase >= 3 else 0
            PH3 = HPC if max_phase >= 4 else 0
            PH4 = 4 if max_phase >= 5 else 0

            # ---- Phase 1.5: qsum and t_rev per head ----
            qsum_sb = consts.tile([128, 2], F32R, name="qsum_sb")
            nc.vector.reduce_sum(qsum_sb[:], qT_sb[:], axis=mybir.AxisListType.X)
            for h in range(PH15):
                p0 = (h % 2) * 64
                for yc in range(8):
                    ps = pp.tile([128, 512], F32, name="ps_t", tag="psA")
                    nc.tensor.matmul(
                        ps[0:1, :], qsum_sb[p0:p0 + 64, h // 2:h // 2 + 1],
                        PTWT_sb[p0:p0 + 64, yc * 512:(yc + 1) * 512],
                        start=True, stop=True)
                    tpiece = work.tile([1, 512], F16, name="tpiece")
                    nc.vector.tensor_copy(out=tpiece[:], in_=ps[0:1, :])
                    nc.sync.dma_start(
                        bass.AP(tensor=trev_dram[h], offset=yc * 512,
                                ap=[[512, 1], [1, 512]]), tpiece[0:1, :])

            # ---- Phase 2: kp windows per head -> DRAM (banded) ----
            # The pos-table clip saturates outside W in [1536, 2560): there
            # the window value is a per-key constant (PTW col 1536 or 2559
            # dotted with k).  Matmul only the interior band; fill the rest
            # by per-partition broadcast on the otherwise-idle gpsimd engine.
            # ptst is dead after the PTWT_sb conversion — zero it and use it
            # as the broadcast-add's zero operand.
            nc.vector.memset(ptst[:], 0.0)
            zbf = ptst
            for h in range(PH2):
                p0 = (h % 2) * 64
                for side, wcol in ((0, 1536), (1, 2559)):
                    for ch in range(4):
                        ps = pp.tile([128, 512], F32, name="ps_kpc", tag="psA")
                        nc.tensor.matmul(
                            ps[0:1, :], PTWT_sb[p0:p0 + 64, wcol:wcol + 1],
                            kT_sb[p0:p0 + 64, h // 2, ch * 512:(ch + 1) * 512],
                            start=True, stop=True)
                        kv = work.tile([1, 512], F32, name="kvrow",
                                       tag="tpiece")
                        nc.vector.tensor_copy(out=kv[:], in_=ps[0:1, :])
                        nc.sync.dma_start(
                            bass.AP(tensor=kpc_dram[h],
                                    offset=side * S + ch * 512,
                                    ap=[[512, 1], [1, 512]]), kv[0:1, :])
            for h in range(PH2):
                p0 = (h % 2) * 64
                kpcv = stage.tile([128, 16, 2], F32, name="kpcv")
                for side in range(2):
                    nc.sync.dma_start(
                        kpcv[:, :, side],
                        bass.AP(tensor=kpc_dram[h], offset=side * S,
                                ap=[[1, 128], [128, 16]]))
                for jc in range(16):
                    wlo = max(0, 1536 - 128 * jc)
                    whi = min(WW, 2560 - 128 * jc)
                    kpw_sb = work.tile([128, WW], BF16, name="kpw_sb")
                    if wlo > 0:
                        nc.gpsimd.tensor_scalar_add(
                            kpw_sb[:, 0:wlo], zbf[:, 0:wlo], kpcv[:, jc, 0:1])
                    if whi < WW:
                        nc.gpsimd.tensor_scalar_add(
                            kpw_sb[:, whi:WW], zbf[:, 0:WW - whi],
                            kpcv[:, jc, 1:2])
                    lhsT = kT_sb[p0:p0 + 64, h // 2, jc * 128:(jc + 1) * 128]
                    w0 = wlo
                    while w0 < whi:
                        wid = min(512, whi - w0)
                        ps = pp.tile([128, 512], F32, name="ps_kp", tag="psA")
                        nc.tensor.matmul(
                            ps[:, :wid], lhsT,
                            PTWT_sb[p0:p0 + 64, 128 * jc + w0:128 * jc + w0 + wid],
                            start=True, stop=True)
                        nc.vector.tensor_copy(out=kpw_sb[:, w0:w0 + wid],
                                              in_=ps[:, :wid])
                        w0 += wid
                    nc.sync.dma_start(kpwin_dram[h][jc], kpw_sb[:])

            # ---- Phase 3: attention per head ----
            aoT_sb = big.tile([128, 2, S], F32R, name="aoT_sb")
            if max_phase < 5:
                zst = work.tile([128, 512], F32, name="ostage")
                nc.vector.memset(zst[:], 0.0)
                nc.vector.tensor_copy(out=aoT_sb[:, 0, 0:512],
                                      in_=zst[:].bitcast(F32R))
            for h in range(PH3):
                p0 = (h % 2) * 64
                TS2 = stage.tile([128, TSW], F16, name="TS2")
                nc.sync.dma_start(
                    TS2[:], bass.AP(tensor=trev_dram[h], offset=0,
                                    ap=[[1, 128], [1, TSW]]))
                avps = [pav.tile([65, 512], F32, name=f"avp{i}", tag=f"avp{i}")
                        for i in range(4)]
                for jc in range(16):
                    # one skewed read serves all 4 query stripes
                    p2c_nat = work.tile([128, 2048], BF16, name="p2c_nat")
                    nc.sync.dma_start(
                        p2c_nat[:],
                        bass.AP(tensor=kpwin_dram[h], offset=jc * 128 * WW,
                                ap=[[WW + 1, 128], [1, 2048]]))
                    for istripe in range(4):
                        sc = pp.tile([128, 512], F32, name="sc", tag="psA")
                        nc.tensor.matmul(
                            sc[:], kT_sb[p0:p0 + 64, h // 2, jc * 128:(jc + 1) * 128],
                            qT_sb[p0:p0 + 64, h // 2, istripe * 512:(istripe + 1) * 512],
                            start=True, stop=True)
                        base = 512 * istripe - 128 * jc + 2048
                        c2p_in = bass.AP(
                            tensor=TS2.tensor,
                            offset=TS2.offset + (4095 - base),
                            ap=[[TSW, 128], [-1, 512]])
                        ssum = work.tile([128, 512], F32, name="ssum")
                        nc.vector.tensor_tensor(
                            ssum[:], sc[:],
                            p2c_nat[:, istripe * 512:(istripe + 1) * 512],
                            mybir.AluOpType.add)
                        ssum2 = work.tile([128, 512], F32, name="ssum2")
                        nc.gpsimd.tensor_tensor(ssum2[:], ssum[:], c2p_in,
                                                mybir.AluOpType.add)
                        sT = work.tile([128, 512], F32R, name="sT")
                        nc.scalar.activation(
                            out=sT[:], in_=ssum2[:],
                            func=mybir.ActivationFunctionType.Exp, scale=0.125)
                        nc.tensor.matmul(avps[istripe][:], v_sb[:, jc, h, :],
                                         sT[:],
                                         start=(jc == 0), stop=(jc == 15))
                for istripe in range(4):
                    av_sb = work.tile([65, 512], F32, name="av_sb")
                    nc.vector.tensor_copy(out=av_sb[:], in_=avps[istripe][:])
                    rec = work.tile([1, 512], F32R, name="rec")
                    nc.vector.reciprocal(out=rec[:], in_=av_sb[64:65, :])
                    rbc = pp.tile([128, 512], F32, name="rbc", tag="psA")
                    nc.tensor.matmul(rbc[0:64, :], onesrow_r[:], rec[:],
                                     start=True, stop=True)
                    nc.vector.tensor_tensor(
                        aoT_sb[p0:p0 + 64, h // 2,
                               istripe * 512:(istripe + 1) * 512],
                        av_sb[0:64, :], rbc[0:64, :], mybir.AluOpType.mult)

            # ---- Phase 4: c_proj partial -> bf16 -> ReduceScatter ----
            for rc in range(PH4):
                for ec in range(8):
                    ps = pp.tile([128, 512], F32, name="ps_o", tag="psA")
                    for cc in range(2):
                        nc.tensor.matmul(
                            ps[:], WcT_sb[:, cc, ec * 128:(ec + 1) * 128],
                            aoT_sb[:, cc, rc * 512:(rc + 1) * 512],
                            start=(cc == 0), stop=(cc == 1))
                    obf = work.tile([128, 512], BF16, name="obf")
                    nc.vector.tensor_copy(out=obf[:], in_=ps[:])
                    nc.sync.dma_start(
                        outPart[ec * 128:(ec + 1) * 128,
                                rc * 512:(rc + 1) * 512], obf[:])
            if max_phase >= 5:
                nc.gpsimd.collective_compute(
                    "ReduceScatter", mybir.AluOpType.add,
                    replica_groups=[[0, 1, 2, 3], [4, 5, 6, 7]],
                    ins=[outPart.ap().opt()], outs=[outRS.ap().opt()])
                nc.sync.dma_start(outP[:], outRS[:])
            else:
                zb = work.tile([128, S], BF16, name="ob", tag="ob")
                nc.vector.memset(zb[:], 0.0)
                nc.sync.dma_start(outP[0:128, :], zb[:])
    nc.compile()
    return nc


_NC_CACHE = None


def _get_nc():
    global _NC_CACHE
    if _NC_CACHE is None:
        _NC_CACHE = build_nc()
    return _NC_CACHE


def _build_in_maps(hidden_states, Wq, bq, Wk, bk, Wv, bv, Wc, pos_table):
    hidden_states = np.asarray(hidden_states, dtype=np.float32)
    Wq, Wk, Wv, Wc = (np.asarray(x, dtype=np.float32) for x in (Wq, Wk, Wv, Wc))
    bq, bk, bv = (np.asarray(x, dtype=np.float32) for x in (bq, bk, bv))
    pos_table = np.asarray(pos_table, dtype=np.float32)

    # raw transposed pos table; the 8x/clip/reverse expansion happens on
    # device (saves shipping the 4096-column table)
    hsT = [np.ascontiguousarray(hidden_states[b].T).astype(NPBF16)
           for b in range(B)]
    ptT_bf = np.ascontiguousarray(pos_table.T).astype(NPBF16)  # [64, 1024]

    # per-head-group weight stream: WqT | WkT | WvT | WcT | bq | bk | bv
    wstream = []
    for i in range(NCORES // B):
        rows = slice(i * CLOC, (i + 1) * CLOC)
        wstream.append(np.concatenate([
            np.ascontiguousarray(Wq[rows].T).ravel(),
            np.ascontiguousarray(Wk[rows].T).ravel(),
            np.ascontiguousarray(Wv[rows].T).ravel(),
            np.ascontiguousarray(Wc[:, rows].T).ravel(),
            bq[rows], bk[rows], bv[rows],
        ]).astype(NPBF16))

    in_maps = []
    for c in range(NCORES):
        b = c // (NCORES // B)
        i = c % (NCORES // B)
        rank = c // 4  # rank within pair [i, i+4]
        pack = np.concatenate([
            hsT[b][i * CLOC:(i + 1) * CLOC].ravel(),
            ptT_bf[16 * i:16 * (i + 1)].ravel(),
            wstream[i][rank * KW:(rank + 1) * KW],
        ])
        in_maps.append(dict(pack=pack))
    return in_maps


def kernel(hidden_states, Wq, bq, Wk, bk, Wv, bv, Wc, pos_table):
    in_maps = _build_in_maps(hidden_states, Wq, bq, Wk, bk, Wv, bv, Wc,
                             pos_table)
    nc = _get_nc()
    results = run_bass_kernel_spmd(nc, in_maps, core_ids=list(range(NCORES)))

    out = np.empty((B, S, D), dtype=np.float32)
    for c in range(NCORES):
        b = c // (NCORES // B)
        i = c % (NCORES // B)
        out[b, :, i * CLOC:(i + 1) * CLOC] = (
            results.results[c]["outP"].T.astype(np.float32))
    return out


# revision 48
# speedup vs baseline: 1.1063x; 1.1063x over previous
"""Trainium2 Bass kernel for DisentangledSelfAttention (DeBERTa-style).

Shapes (hardcoded): B=2, S=2048, D=1024, H=16, Dh=64, MAX_REL=512.

Sharding: 8 cores; core c handles batch b = c//4 and heads h0 = (c%4)*4 .. +4
(tensor-parallel on heads for q/k/v columns and c_proj rows; data-parallel on
batch).

Host<->device traffic is minimized (the axon tunnel is the bottleneck):
  - Each core receives only 1/4 of its batch's transposed hidden states plus
    1/4 of the position table (bf16, ~1.1MB) and HALF of its weight slice
    (bf16, ~1.0MB).  On-device AllGathers reassemble them: hs+postable over
    batch groups [[0..3],[4..7]], weights over pairs [[0,4],[1,5],[2,6],[3,7]]
    (a pair shares the same head slice, so the two halves combine).
  - Compute runs in f32r exactly as before (bf16 operands are upcast on load).
  - The per-core c_proj partial [1024, 2048] is ReduceScattered (add, bf16)
    over the batch group, so each core returns only its 256 output channels
    [256, 2048] bf16; the host transposes/concats.

Math per core (heads are local 0..3):
  qT/kT [256, 2048] = W.T-slice @ hsT (+bias), v [2048, 256] natural.
  scoresT[j,i] = k_h.T q_h + 8*t[clip(i-j+512)] + 8*kp-term, exp'd with
  scale 1/8, then out = (v|1).T @ exp  -> av[65, i], normalized by row 64.
  c2p uses t = qsum @ PTW (Toeplitz band added via overlap-staged TS2 tile
  read with a negative-free-stride add on gpsimd); p2c uses per-(head,jc)
  kp windows in anti-diagonal coords, bounced through DRAM and re-read with
  a skewed (diagonal) DMA access pattern covering all 4 query stripes at
  once, then added on the vector engine (the two identity-select matmuls
  these adds replace cost ~2x the score matmul on this backend).  The kp
  matmul only covers the non-saturated band W in [1536, 2560) of
  PTW[w] = 8*pos_table[clip(2559-w)]; outside it the window is the per-key
  constant k.PTW[1536 or 2559], broadcast-filled by gpsimd tensor_scalar
  (cuts phase-2 tensor-engine output elements by 57%).  Four PSUM
  accumulators run the AV matmuls for all query stripes concurrently.
"""
import os

os.environ.setdefault("NEURON_RT_RESET_CORES", "1")

import numpy as np
import ml_dtypes

import concourse.bass as bass
import concourse.bacc as bacc
import concourse.mybir as mybir
import concourse.tile as tile
from concourse.bass_utils import run_bass_kernel_spmd

F32, BF16, F32R = mybir.dt.float32, mybir.dt.bfloat16, mybir.dt.float32r
F16, I8 = mybir.dt.float16, mybir.dt.int8
NPBF16 = ml_dtypes.bfloat16

B, S, D = 2, 2048, 1024
H, Dh, MAX_REL = 16, 64, 512
NCORES = 8
HPC = H // (NCORES // B)   # heads per core = 4
CLOC = HPC * Dh            # local head-dim columns = 256
WW = 2176                  # kp window width per jc chunk
TSW = 3968                 # c2p staging width
PTWN = 4096                # table rows

# --- packed-input layout (element counts, bf16) ---
HSQ = 256 * S              # hs quarter = 524288
PTQ = 16 * 1024            # raw pos-table quarter (16 of 64 ptT rows) = 16384
AGH = HSQ + PTQ            # per-member stride in gathered hs pack = 540672
NUH = 4 * AGH              # gathered hs pack

WSL = D * CLOC             # one weight slice = 262144
NW = 4 * WSL + 3 * CLOC    # weights + 3 bias slices = 1049344
KW = NW // 2               # per-core half = 524672
OFF_WK, OFF_WV, OFF_WC = WSL, 2 * WSL, 3 * WSL
OFF_B = 4 * WSL


def build_nc(max_phase=9):
    nc = bacc.Bacc("TRN2", target_bir_lowering=False)
    pack = nc.dram_tensor("pack", [AGH + KW], BF16, kind="ExternalInput")
    outP = nc.dram_tensor("outP", [CLOC, S], BF16, kind="ExternalOutput")

    aginh = nc.dram_tensor("aginh", [AGH], BF16, kind="Internal")
    aginw = nc.dram_tensor("aginw", [KW], BF16, kind="Internal")
    Uhs = nc.dram_tensor("Uhs", [NUH], BF16, kind="Internal")
    UW = nc.dram_tensor("UW", [NW], BF16, kind="Internal")
    outPart = nc.dram_tensor("outPart", [D, S], BF16, kind="Internal")
    outRS = nc.dram_tensor("outRS", [CLOC, S], BF16, kind="Internal")

    trev_dram = [nc.dram_tensor(f"trev{h}", [PTWN], F16, kind="Internal")
                 for h in range(HPC)]
    # per-head clip-constant rows: kpc[side, j] = 8*k[j].pt[1023 or 0]
    kpc_dram = [nc.dram_tensor(f"kpc{h}", [2, S], F32, kind="Internal")
                for h in range(HPC)]
    kpwin_dram = [nc.dram_tensor(f"kpwin{h}", [16, 128, WW], BF16,
                                 kind="Internal") for h in range(HPC)]

    with tile.TileContext(nc) as tc:
        with (
            tc.tile_pool(name="consts", bufs=1) as consts,
            tc.tile_pool(name="big", bufs=1) as big,
            tc.tile_pool(name="work", bufs=2) as work,
            tc.tile_pool(name="stage", bufs=1) as stage,
            tc.tile_pool(name="hsst", bufs=2) as hsst,
            tc.tile_pool(name="wst", bufs=2) as wstp,
            tc.tile_pool(name="pp", bufs=4, space="PSUM") as pp,
            tc.tile_pool(name="pav", bufs=1, space="PSUM") as pav,
            nc.allow_low_precision(reason="f32r operand rounding throughout"),
        ):
            # ---- Phase -1: bounce packed inputs, AllGather on device ----
            nc.sync.dma_start(aginh[:], pack[0:AGH])
            nc.sync.dma_start(aginw[:], pack[AGH:AGH + KW])
            nc.gpsimd.collective_compute(
                "AllGather", mybir.AluOpType.bypass,
                replica_groups=[[0, 4], [1, 5], [2, 6], [3, 7]],
                ins=[aginw.ap().opt()], outs=[UW.ap().opt()])
            nc.gpsimd.collective_compute(
                "AllGather", mybir.AluOpType.bypass,
                replica_groups=[[0, 1, 2, 3], [4, 5, 6, 7]],
                ins=[aginh.ap().opt()], outs=[Uhs.ap().opt()])

            # ---- Phase 0: constants / weights / tables (bf16 -> f32r) ----
            WqT_sb = consts.tile([128, 8, CLOC], F32R, name="WqT_sb")
            WkT_sb = consts.tile([128, 8, CLOC], F32R, name="WkT_sb")
            WvT_sb = consts.tile([128, 8, CLOC], F32R, name="WvT_sb")
            for dst, off in ((WqT_sb, 0), (WkT_sb, OFF_WK), (WvT_sb, OFF_WV)):
                wt = wstp.tile([128, 8, CLOC], BF16, name="wt", tag="wt")
                nc.sync.dma_start(
                    wt[:], bass.AP(tensor=UW, offset=off,
                                   ap=[[CLOC, 128], [128 * CLOC, 8], [1, CLOC]]))
                nc.vector.tensor_copy(out=dst[:], in_=wt[:])
            WcT_sb = consts.tile([128, 2, D], F32R, name="WcT_sb")
            wt = wstp.tile([128, 2, D], BF16, name="wtc", tag="wt")
            nc.sync.dma_start(
                wt[:], bass.AP(tensor=UW, offset=OFF_WC,
                               ap=[[D, 128], [128 * D, 2], [1, D]]))
            nc.vector.tensor_copy(out=WcT_sb[:], in_=wt[:])

            # Build PTWT[d, W] = 8*ptT[d, clip(2559-W, 0, 1023)] on device
            # from raw bf16 ptT quarters (saves 96KB/core of transfer):
            # W in [0, 1537) -> const col 1023; [1537, 2560) -> reversed
            # slice; [2560, 4096) -> const col 0.
            PTWT_sb = consts.tile([128, PTWN], F32R, name="PTWT_sb")
            ptst = consts.tile([128, 1600], BF16, name="ptst")
            for k in range(4):
                src = bass.AP(tensor=Uhs, offset=k * AGH + HSQ,
                              ap=[[1024, 16], [1, 1024]])
                nc.sync.dma_start(ptst[16 * k:16 * (k + 1), 0:1024], src)
                nc.sync.dma_start(ptst[64 + 16 * k:64 + 16 * (k + 1), 0:1024],
                                  src)
            pt8 = consts.tile([128, 1024], F32R, name="pt8")
            nc.scalar.activation(
                out=pt8[:], in_=ptst[:, 0:1024],
                func=mybir.ActivationFunctionType.Identity, scale=8.0)
            for a, b, bias_col in ((0, 1024, 1023), (1024, 1537, 1023),
                                   (2560, 3584, 0), (3584, 4096, 0)):
                nc.scalar.activation(
                    out=PTWT_sb[:, a:b], in_=pt8[:, 0:b - a],
                    func=mybir.ActivationFunctionType.Identity,
                    bias=pt8[:, bias_col:bias_col + 1], scale=0.0)
            nc.vector.tensor_copy(
                out=PTWT_sb[:, 1537:2560],
                in_=bass.AP(tensor=pt8.tensor, offset=pt8.offset + 1022,
                            ap=[[1024, 128], [-1, 1023]]))

            bq_sb = consts.tile([128, 2], F32, name="bq_sb")
            bk_sb = consts.tile([128, 2], F32, name="bk_sb")
            bst = consts.tile([128, 2, 2], BF16, name="bst")
            for i, dst in enumerate((bq_sb, bk_sb)):
                nc.sync.dma_start(
                    bst[:, :, i], bass.AP(tensor=UW, offset=OFF_B + i * CLOC,
                                          ap=[[1, 128], [128, 2]]))
                nc.vector.tensor_copy(out=dst[:], in_=bst[:, :, i])
            bv_bc = consts.tile([128, CLOC], F32, name="bv_bc")
            bvst = consts.tile([128, CLOC], BF16, name="bvst")
            nc.sync.dma_start(
                bvst[:], bass.AP(tensor=UW, offset=OFF_B + 2 * CLOC,
                                 ap=[[0, 128], [1, CLOC]]))
            nc.vector.tensor_copy(out=bv_bc[:], in_=bvst[:])

            ones_f = consts.tile([128, 1], F32, name="ones_f")
            nc.vector.memset(ones_f[:], 1.0)
            ones_r = consts.tile([128, 1], F32R, name="ones_r")
            nc.vector.tensor_copy(out=ones_r[:], in_=ones_f[:])
            onesrow_f = consts.tile([1, 64], F32, name="onesrow_f")
            nc.vector.memset(onesrow_f[:], 1.0)
            onesrow_r = consts.tile([1, 64], F32R, name="onesrow_r")
            nc.vector.tensor_copy(out=onesrow_r[:], in_=onesrow_f[:])

            # ---- Phase 1: projections, streaming hs in 256-col chunks ----
            qT_sb = big.tile([128, 2, S], F32R, name="qT_sb")
            kT_sb = big.tile([128, 2, S], F32R, name="kT_sb")
            v_sb = big.tile([128, 16, HPC, 65], F32R, name="v_sb")
            for rc in range(8):
                r0 = rc * 256
                hs_bf = hsst.tile([128, 8, 256], BF16, name="hs_bf", tag="hsbf")
                for k in range(4):
                    nc.sync.dma_start(
                        hs_bf[:, 2 * k:2 * k + 2, :],
                        bass.AP(tensor=Uhs, offset=k * AGH + r0,
                                ap=[[S, 128], [128 * S, 2], [1, 256]]))
                hs_ck = hsst.tile([128, 8, 256], F32R, name="hs_ck", tag="hsck")
                nc.vector.tensor_copy(out=hs_ck[:], in_=hs_bf[:])
                for dst, w_sb, b_sb in ((qT_sb, WqT_sb, bq_sb),
                                        (kT_sb, WkT_sb, bk_sb)):
                    for hh in range(2):
                        ps = pp.tile([128, 512], F32, name="ps_proj", tag="psA")
                        for dc in range(8):
                            nc.tensor.matmul(
                                ps[:, 0:256],
                                w_sb[:, dc, hh * 128:(hh + 1) * 128],
                                hs_ck[:, dc, :],
                                start=(dc == 0), stop=(dc == 7))
                        nc.scalar.activation(
                            out=dst[:, hh, r0:r0 + 256], in_=ps[:, 0:256],
                            func=mybir.ActivationFunctionType.Identity,
                            bias=b_sb[:, hh:hh + 1], scale=1.0)
                for sub in range(2):
                    rr = rc * 2 + sub
                    ps = pp.tile([128, 512], F32, name="ps_v", tag="psA")
                    for dc in range(8):
                        nc.tensor.matmul(
                            ps[:, 0:256], hs_ck[:, dc, sub * 128:(sub + 1) * 128],
                            WvT_sb[:, dc, :], start=(dc == 0), stop=(dc == 7))
                    for h in range(HPC):
                        nc.vector.tensor_tensor(
                            v_sb[:, rr, h, 0:64], ps[:, h * 64:(h + 1) * 64],
                            bv_bc[:, h * 64:(h + 1) * 64], mybir.AluOpType.add)
                        nc.vector.tensor_copy(out=v_sb[:, rr, h, 64:65],
                                              in_=ones_r[:])

            # phase gating for bisection
            PH15 = HPC if max_phase >= 2 else 0
            PH2 = HPC if max_phase >= 3 else 0
            PH3 = HPC if max_phase >= 4 else 0
            PH4 = 4 if max_phase >= 5 else 0

            # ---- Phase 1.5: qsum and t_rev per head ----
            qsum_sb = consts.tile([128, 2], F32R, name="qsum_sb")
            nc.vector.reduce_sum(qsum_sb[:], qT_sb[:], axis=mybir.AxisListType.X)
            for h in range(PH15):
                p0 = (h % 2) * 64
                for yc in range(8):
                    ps = pp.tile([128, 512], F32, name="ps_t", tag="psA")
                    nc.tensor.matmul(
                        ps[0:1, :], qsum_sb[p0:p0 + 64, h // 2:h // 2 + 1],
                        PTWT_sb[p0:p0 + 64, yc * 512:(yc + 1) * 512],
                        start=True, stop=True)
                    tpiece = work.tile([1, 512], F16, name="tpiece")
                    nc.vector.tensor_copy(out=tpiece[:], in_=ps[0:1, :])
                    nc.sync.dma_start(
                        bass.AP(tensor=trev_dram[h], offset=yc * 512,
                                ap=[[512, 1], [1, 512]]), tpiece[0:1, :])

            # ---- Phase 2: kp windows per head -> DRAM (banded) ----
            # The pos-table clip saturates outside W in [1536, 2560): there
            # the window value is a per-key constant (PTW col 1536 or 2559
            # dotted with k).  Matmul only the interior band; fill the rest
            # by per-partition broadcast on the otherwise-idle gpsimd engine.
            # ptst is dead after the PTWT_sb conversion — zero it and use it
            # as the broadcast-add's zero operand.
            nc.vector.memset(ptst[:], 0.0)
            zbf = ptst
            for h in range(PH2):
                p0 = (h % 2) * 64
                for side, wcol in ((0, 1536), (1, 2559)):
                    for ch in range(4):
                        ps = pp.tile([128, 512], F32, name="ps_kpc", tag="psA")
                        nc.tensor.matmul(
                            ps[0:1, :], PTWT_sb[p0:p0 + 64, wcol:wcol + 1],
                            kT_sb[p0:p0 + 64, h // 2, ch * 512:(ch + 1) * 512],
                            start=True, stop=True)
                        kv = work.tile([1, 512], F32, name="kvrow",
                                       tag="tpiece")
                        nc.vector.tensor_copy(out=kv[:], in_=ps[0:1, :])
                        nc.sync.dma_start(
                            bass.AP(tensor=kpc_dram[h],
                                    offset=side * S + ch * 512,
                                    ap=[[512, 1], [1, 512]]), kv[0:1, :])
            for h in range(PH2):
                p0 = (h % 2) * 64
                kpcv = stage.tile([128, 16, 2], F32, name="kpcv")
                for side in range(2):
                    nc.sync.dma_start(
                        kpcv[:, :, side],
                        bass.AP(tensor=kpc_dram[h], offset=side * S,
                                ap=[[1, 128], [128, 16]]))
                for jc in range(16):
                    wlo = max(0, 1536 - 128 * jc)
                    whi = min(WW, 2560 - 128 * jc)
                    kpw_sb = work.tile([128, WW], BF16, name="kpw_sb")
                    if wlo > 0:
                        nc.gpsimd.tensor_scalar_add(
                            kpw_sb[:, 0:wlo], zbf[:, 0:wlo], kpcv[:, jc, 0:1])
                    if whi < WW:
                        nc.gpsimd.tensor_scalar_add(
                            kpw_sb[:, whi:WW], zbf[:, 0:WW - whi],
                            kpcv[:, jc, 1:2])
                    lhsT = kT_sb[p0:p0 + 64, h // 2, jc * 128:(jc + 1) * 128]
                    w0 = wlo
                    while w0 < whi:
                        wid = min(512, whi - w0)
                        ps = pp.tile([128, 512], F32, name="ps_kp", tag="psA")
                        nc.tensor.matmul(
                            ps[:, :wid], lhsT,
                            PTWT_sb[p0:p0 + 64, 128 * jc + w0:128 * jc + w0 + wid],
                            start=True, stop=True)
                        nc.vector.tensor_copy(out=kpw_sb[:, w0:w0 + wid],
                                              in_=ps[:, :wid])
                        w0 += wid
                    nc.sync.dma_start(kpwin_dram[h][jc], kpw_sb[:])

            # ---- Phase 3: attention per head ----
            aoT_sb = big.tile([128, 2, S], F32R, name="aoT_sb")
            if max_phase < 5:
                zst = work.tile([128, 512], F32, name="ostage")
                nc.vector.memset(zst[:], 0.0)
                nc.vector.tensor_copy(out=aoT_sb[:, 0, 0:512],
                                      in_=zst[:].bitcast(F32R))
            for h in range(PH3):
                p0 = (h % 2) * 64
                TS2 = stage.tile([128, TSW], F16, name="TS2")
                nc.sync.dma_start(
                    TS2[:], bass.AP(tensor=trev_dram[h], offset=0,
                                    ap=[[1, 128], [1, TSW]]))
                avps = [pav.tile([65, 512], F32, name=f"avp{i}", tag=f"avp{i}")
                        for i in range(4)]
                for jc in range(16):
                    # one skewed read serves all 4 query stripes
                    p2c_nat = work.tile([128, 2048], BF16, name="p2c_nat")
                    nc.sync.dma_start(
                        p2c_nat[:],
                        bass.AP(tensor=kpwin_dram[h], offset=jc * 128 * WW,
                                ap=[[WW + 1, 128], [1, 2048]]))
                    for istripe in range(4):
                        sc = pp.tile([128, 512], F32, name="sc", tag="psA")
                        nc.tensor.matmul(
                            sc[:], kT_sb[p0:p0 + 64, h // 2, jc * 128:(jc + 1) * 128],
                            qT_sb[p0:p0 + 64, h // 2, istripe * 512:(istripe + 1) * 512],
                            start=True, stop=False)
                        base = 512 * istripe - 128 * jc + 2048
                        c2p_in = bass.AP(
                            tensor=TS2.tensor,
                            offset=TS2.offset + (4095 - base),
                            ap=[[TSW, 128], [-1, 512]])
                        ssum = work.tile([128, 512], F32, name="ssum")
                        nc.vector.tensor_tensor(
                            ssum[:], sc[:],
                            p2c_nat[:, istripe * 512:(istripe + 1) * 512],
                            mybir.AluOpType.add)
                        ssum2 = work.tile([128, 512], F32, name="ssum2")
                        nc.gpsimd.tensor_tensor(ssum2[:], ssum[:], c2p_in,
                                                mybir.AluOpType.add)
                        sT = work.tile([128, 512], F32R, name="sT")
                        nc.scalar.activation(
                            out=sT[:], in_=ssum2[:],
                            func=mybir.ActivationFunctionType.Exp, scale=0.125)
                        nc.tensor.matmul(avps[istripe][:], v_sb[:, jc, h, :],
                                         sT[:],
                                         start=(jc == 0), stop=(jc == 15))
                for istripe in range(4):
                    av_sb = work.tile([65, 512], F32, name="av_sb")
                    nc.vector.tensor_copy(out=av_sb[:], in_=avps[istripe][:])
                    rec = work.tile([1, 512], F32R, name="rec")
                    nc.vector.reciprocal(out=rec[:], in_=av_sb[64:65, :])
                    rbc = pp.tile([128, 512], F32, name="rbc", tag="psA")
                    nc.tensor.matmul(rbc[0:64, :], onesrow_r[:], rec[:],
                                     start=True, stop=True)
                    nc.vector.tensor_tensor(
                        aoT_sb[p0:p0 + 64, h // 2,
                               istripe * 512:(istripe + 1) * 512],
                        av_sb[0:64, :], rbc[0:64, :], mybir.AluOpType.mult)

            # ---- Phase 4: c_proj partial -> bf16 -> ReduceScatter ----
            for rc in range(PH4):
                for ec in range(8):
                    ps = pp.tile([128, 512], F32, name="ps_o", tag="psA")
                    for cc in range(2):
                        nc.tensor.matmul(
                            ps[:], WcT_sb[:, cc, ec * 128:(ec + 1) * 128],
                            aoT_sb[:, cc, rc * 512:(rc + 1) * 512],
                            start=(cc == 0), stop=(cc == 1))
                    obf = work.tile([128, 512], BF16, name="obf")
                    nc.vector.tensor_copy(out=obf[:], in_=ps[:])
                    nc.sync.dma_start(
                        outPart[ec * 128:(ec + 1) * 128,
                                rc * 512:(rc + 1) * 512], obf[:])
            if max_phase >= 5:
                nc.gpsimd.collective_compute(
                    "ReduceScatter", mybir.AluOpType.add,
                    replica_groups=[[0, 1, 2, 3], [4, 5, 6, 7]],
                    ins=[outPart.ap().opt()], outs=[outRS.ap().opt()])
                nc.sync.dma_start(outP[:], outRS[:])
            else:
                zb = work.tile([128, S], BF16, name="ob", tag="ob")
                nc.vector.memset(zb[:], 0.0)
                nc.sync.dma_start(outP[0:128, :], zb[:])
    nc.compile()
    return nc


_NC_CACHE = None


def _get_nc():
    global _NC_CACHE
    if _NC_CACHE is None:
        _NC_CACHE = build_nc()
    return _NC_CACHE


def _build_in_maps(hidden_states, Wq, bq, Wk, bk, Wv, bv, Wc, pos_table):
    hidden_states = np.asarray(hidden_states, dtype=np.float32)
    Wq, Wk, Wv, Wc = (np.asarray(x, dtype=np.float32) for x in (Wq, Wk, Wv, Wc))
    bq, bk, bv = (np.asarray(x, dtype=np.float32) for x in (bq, bk, bv))
    pos_table = np.asarray(pos_table, dtype=np.float32)

    # raw transposed pos table; the 8x/clip/reverse expansion happens on
    # device (saves shipping the 4096-column table)
    hsT = [np.ascontiguousarray(hidden_states[b].T).astype(NPBF16)
           for b in range(B)]
    ptT_bf = np.ascontiguousarray(pos_table.T).astype(NPBF16)  # [64, 1024]

    # per-head-group weight stream: WqT | WkT | WvT | WcT | bq | bk | bv
    wstream = []
    for i in range(NCORES // B):
        rows = slice(i * CLOC, (i + 1) * CLOC)
        wstream.append(np.concatenate([
            np.ascontiguousarray(Wq[rows].T).ravel(),
            np.ascontiguousarray(Wk[rows].T).ravel(),
            np.ascontiguousarray(Wv[rows].T).ravel(),
            np.ascontiguousarray(Wc[:, rows].T).ravel(),
            bq[rows], bk[rows], bv[rows],
        ]).astype(NPBF16))

    in_maps = []
    for c in range(NCORES):
        b = c // (NCORES // B)
        i = c % (NCORES // B)
        rank = c // 4  # rank within pair [i, i+4]
        pack = np.concatenate([
            hsT[b][i * CLOC:(i + 1) * CLOC].ravel(),
            ptT_bf[16 * i:16 * (i + 1)].ravel(),
            wstream[i][rank * KW:(rank + 1) * KW],
        ])
        in_maps.append(dict(pack=pack))
    return in_maps


def kernel(hidden_states, Wq, bq, Wk, bk, Wv, bv, Wc, pos_table):
    in_maps = _build_in_maps(hidden_states, Wq, bq, Wk, bk, Wv, bv, Wc,
                             pos_table)
    nc = _get_nc()
    results = run_bass_kernel_spmd(nc, in_maps, core_ids=list(range(NCORES)))

    out = np.empty((B, S, D), dtype=np.float32)
    for c in range(NCORES):
        b = c // (NCORES // B)
        i = c % (NCORES // B)
        out[b, :, i * CLOC:(i + 1) * CLOC] = (
            results.results[c]["outP"].T.astype(np.float32))
    return out
